# revision 22
# baseline (speedup 1.0000x reference)
"""Bass/Trainium2 kernel for nn_CrossAttentionBlock (B=2, T=2048, D=1024, H=16).

Sharding: 8 cores; core c owns heads {2c, 2c+1} for BOTH batches (tensor
parallel over heads).  Output rows: core c owns q-rows [c*256,(c+1)*256) of
BOTH batches; one AllToAll per batch redistributes per-head context, the
first one fully overlapped with batch-1 attention.

Math notes:
  - masked softmax logits: s_eff = s*(1+m)/16 with m in {0,1}
    => unnormalized weights et = select(m, E^2, E) with E = exp(s/16).
    E computed by ScalarE directly from PSUM (scale=1/16 free); E^2 on
    DVE/GPSIMD (bf16 SBUF 2x); select via DVE copy_predicated(mask=m).
  - clamp(+-50) is a provable no-op for these inputs (|s_eff| < ~7).
  - softmax denominator accumulated via a ones-column appended to v.
  - v-bias bv folded into the out-proj bias on host:
    bo' = bo + Wo^T bv  (softmax rows sum to 1).
  - reciprocal via reciprocal_approx_fast; broadcast via DMA (stride-0).
"""

import sys

sys.path.insert(0, "/opt/trn_rl_repo")

from collections import deque
from contextlib import ExitStack

import numpy as np
import ml_dtypes

import concourse.bass as bass
import concourse.mybir as mybir
import concourse.tile as tile
from concourse import bacc
from concourse import tile_utils
from concourse.bass_utils import run_bass_kernel_spmd
from concourse.masks import make_identity

tile_utils.max_sbuf_usage = 204 * 1024

BF16 = mybir.dt.bfloat16
F32 = mybir.dt.float32
AF = mybir.ActivationFunctionType
ALU = mybir.AluOpType
ts = bass.ts

N_CORES = 8
B, D, H = 2, 1024, 16
T = 2048
HD = D // H              # 64 head dim
HPC = H // N_CORES       # 2 heads per core
GW = HPC * HD            # 128 head-group width per core
DC = D // 128            # 8 d chunks
KC = T // 128            # 16 k chunks
NB = 512                 # proj moving N
QH = T // N_CORES        # 256 output q rows per core per batch
NQC = QH // 128          # 2

USE_SELECT = True        # select-path masking (vs baseline fp32 mul path)

_cached = {}


def build_kernel():
    nc = bacc.Bacc(None, num_devices=N_CORES)

    qT_h = nc.dram_tensor("qT", [B, D, T], BF16, kind="ExternalInput")
    kT_h = nc.dram_tensor("kT", [B, D, T], BF16, kind="ExternalInput")
    fT_h = nc.dram_tensor("fT", [B, T, T], BF16, kind="ExternalInput")
    wq_h = nc.dram_tensor("wq", [D, GW], BF16, kind="ExternalInput")
    wk_h = nc.dram_tensor("wk", [D, GW], BF16, kind="ExternalInput")
    wv_h = nc.dram_tensor("wv", [D, GW], BF16, kind="ExternalInput")
    wo_h = nc.dram_tensor("wo", [D, D], BF16, kind="ExternalInput")
    bq_h = nc.dram_tensor("bq", [GW, 1], F32, kind="ExternalInput")
    bk_h = nc.dram_tensor("bk", [GW, 1], F32, kind="ExternalInput")
    bo_h = nc.dram_tensor("bo", [128, DC], F32, kind="ExternalInput")
    gamma_h = nc.dram_tensor("gamma", [D], F32, kind="ExternalInput")
    beta_h = nc.dram_tensor("beta", [D], F32, kind="ExternalInput")
    qres_h = {}
    out_h = {}
    for b in range(B):
        qres_h[b] = nc.dram_tensor(f"qres{b}", [QH, D], F32, kind="ExternalInput")
        out_h[b] = nc.dram_tensor(f"out{b}", [QH, D], F32, kind="ExternalOutput")

    a2a_in = [nc.dram_tensor(f"a2a_in{b}", [N_CORES, GW, QH], BF16) for b in range(B)]
    a2a_out = [nc.dram_tensor(f"a2a_out{b}", [N_CORES, GW, QH], BF16) for b in range(B)]

    with tile.TileContext(nc) as tc:
        with (
            tc.tile_pool(name="consts", bufs=1) as consts,
            tc.tile_pool(name="ps2", bufs=2, space="PSUM") as ps2_pool,
            tc.tile_pool(name="pspc", bufs=2, space="PSUM") as pc_pool,
            tc.tile_pool(name="psaux", bufs=2, space="PSUM") as aux_pool,
            tc.tile_pool(name="xt", bufs=4) as xt_pool,
            tc.tile_pool(name="qk", bufs=2) as qk_pool,
            tc.tile_pool(name="vpool", bufs=2) as v_pool,
            tc.tile_pool(name="ft", bufs=22) as ft_pool,
            tc.tile_pool(name="et", bufs=4) as et_pool,
            tc.tile_pool(name="sq", bufs=3) as sq_pool,
            tc.tile_pool(name="cm", bufs=4) as cm_pool,
            tc.tile_pool(name="cn", bufs=4) as cn_pool,
            tc.tile_pool(name="sums", bufs=2) as sums_pool,
            tc.tile_pool(name="rb", bufs=4) as rb_pool,
            tc.tile_pool(name="qres", bufs=2) as qres_pool,
            tc.tile_pool(name="ctxt", bufs=8) as ctxt_pool,
            tc.tile_pool(name="outt", bufs=8) as outt_pool,
            tc.tile_pool(name="tail", bufs=2) as tail_pool,
        ):
            # ---------------- constants (startup set) ----------------
            wq_sb = consts.tile([128, DC, GW], BF16, tag="wq")
            nc.sync.dma_start(out=wq_sb, in_=bass.AP(wq_h, 0, [[GW, 128], [128 * GW, DC], [1, GW]]))
            wk_sb = consts.tile([128, DC, GW], BF16, tag="wk")
            nc.sync.dma_start(out=wk_sb, in_=bass.AP(wk_h, 0, [[GW, 128], [128 * GW, DC], [1, GW]]))
            wv_sb = consts.tile([128, DC, GW], BF16, tag="wv")
            nc.sync.dma_start(out=wv_sb, in_=bass.AP(wv_h, 0, [[GW, 128], [128 * GW, DC], [1, GW]]))
            bq_sb = consts.tile([GW, 1], F32, tag="bq")
            nc.sync.dma_start(out=bq_sb, in_=bq_h[:, :])
            bk_sb = consts.tile([GW, 1], F32, tag="bk")
            nc.sync.dma_start(out=bk_sb, in_=bk_h[:, :])
            eps_sb = consts.tile([128, 1], F32, tag="eps")
            nc.vector.memset(eps_sb, 1e-5)
            ones_sb = consts.tile([1, HD], BF16, tag="ones")
            nc.vector.memset(ones_sb, 1.0)

            qT_sb, kT_sb, v_sb = {}, {}, {}

            # ---------------- projection emitters ----------------
            def emit_proj_closures(b):
                """Returns (dma_fn, list-of-closures) for batch b's q/k/v proj."""
                qT_sb[b] = qk_pool.tile([GW, T], BF16, tag="qT", name=f"qT{b}")
                kT_sb[b] = qk_pool.tile([GW, T], BF16, tag="kT", name=f"kT{b}")
                vt = v_pool.tile([128, KC, HPC * (HD + 1)], BF16, tag="v", name=f"v{b}")
                v_sb[b] = vt
                xtq, xtk = [], []

                def dmas():
                    for half in range(4):
                        xh = xt_pool.tile([128, 2, T], BF16, tag="xt", name=f"xtq{b}_{half}")
                        nc.sync.dma_start(
                            out=xh,
                            in_=bass.AP(qT_h, b * D * T + half * 2 * 128 * T,
                                        [[T, 128], [128 * T, 2], [1, T]]),
                        )
                        xtq.append(xh)
                    for half in range(4):
                        xh = xt_pool.tile([128, 2, T], BF16, tag="xt", name=f"xtk{b}_{half}")
                        nc.sync.dma_start(
                            out=xh,
                            in_=bass.AP(kT_h, b * D * T + half * 2 * 128 * T,
                                        [[T, 128], [128 * T, 2], [1, T]]),
                        )
                        xtk.append(xh)

                closures = []
                # q and k projections: per nb, accumulate 8 kc into one aux bank
                for which, xts, w_sb, b_sb, dst in (
                    ("q", xtq, wq_sb, bq_sb, "qT"),
                    ("k", xtk, wk_sb, bk_sb, "kT"),
                ):
                    for nb in range(T // NB):
                        ps_ref = []

                        def mk_mm(kc, nb=nb, xts=xts, w_sb=w_sb, ps_ref=ps_ref,
                                  which=which):
                            def f():
                                if kc == 0:
                                    ps_ref.append(aux_pool.tile(
                                        [128, NB], F32, tag="aux",
                                        name=f"pj{which}{b}_{nb}"))
                                nc.tensor.matmul(
                                    ps_ref[0],
                                    w_sb[:, kc, :],
                                    xts[kc // 2][:, kc % 2, ts(nb, NB)],
                                    start=(kc == 0), stop=(kc == DC - 1),
                                )
                            return f

                        for kc in range(DC):
                            closures.append(mk_mm(kc))

                        def mk_bias(nb=nb, ps_ref=ps_ref, b_sb=b_sb, dst=dst):
                            def f():
                                tgt = qT_sb[b] if dst == "qT" else kT_sb[b]
                                nc.scalar.activation(
                                    tgt[:, ts(nb, NB)], ps_ref[0], AF.Identity,
                                    bias=b_sb[:, :])
                            return f

                        closures.append(mk_bias())

                # v projection: per mc, accumulate 8 kc, then DVE copy (interleaved)
                for mc in range(KC):
                    psv_ref = []

                    def mk_vmm(kc, mc=mc, psv_ref=psv_ref):
                        def f():
                            if kc == 0:
                                psv_ref.append(aux_pool.tile(
                                    [128, NB], F32, tag="aux", name=f"pjv{b}_{mc}"))
                            nc.tensor.matmul(
                                psv_ref[0][:, 0:GW],
                                xtk[kc // 2][:, kc % 2, ts(mc, 128)],
                                wv_sb[:, kc, :],
                                start=(kc == 0), stop=(kc == DC - 1),
                            )
                        return f

                    for kc in range(DC):
                        closures.append(mk_vmm(kc))

                    def mk_vcopy(mc=mc, psv_ref=psv_ref, vt=vt):
                        def f():
                            vslot = vt[:, mc, :]
                            dst = bass.AP(vslot.tensor, vslot.offset,
                                          [vslot.ap[0], [HD + 1, HPC], [1, HD]])
                            psl = psv_ref[0][:, 0:GW]
                            src = bass.AP(psl.tensor, psl.offset,
                                          [psl.ap[0], [HD, HPC], [1, HD]])
                            nc.vector.tensor_copy(dst, src)
                        return f

                    closures.append(mk_vcopy())

                def mk_ones(vt=vt):
                    def f():
                        nc.vector.memset(vt[:, :, HD:HD + 1], 1.0)
                        nc.vector.memset(vt[:, :, 2 * HD + 1:2 * HD + 2], 1.0)
                    return f

                closures.append(mk_ones())
                return dmas, closures

            # ---------------- attention loop ----------------
            ctx_cm = {}

            def emit_norm_pair(b, jqp):
                """Normalize context of jq pair (2*jqp, 2*jqp+1) and DMA to a2a_in."""
                su = sums_pool.tile([4, NB], F32, tag="sums", name=f"su{b}_{jqp}")
                for r, (jq, hl) in enumerate(
                        (jq, hl) for jq in (2 * jqp, 2 * jqp + 1) for hl in range(HPC)):
                    cm = ctx_cm[(jq, hl)]
                    nc.sync.dma_start(out=su[r:r + 1, :], in_=cm[HD:HD + 1, :])
                rc = sums_pool.tile([4, NB], F32, tag="rc", name=f"rc{b}_{jqp}")
                nc.vector.reciprocal_approx_fast(out=rc, in_=su)
                rbf = sums_pool.tile([4, NB], BF16, tag="rbf", name=f"rbf{b}_{jqp}")
                nc.vector.tensor_copy(rbf, rc)
                for r, (jq, hl) in enumerate(
                        (jq, hl) for jq in (2 * jqp, 2 * jqp + 1) for hl in range(HPC)):
                    cm = ctx_cm[(jq, hl)]
                    r1 = rb_pool.tile([1, NB], BF16, tag="r1", name=f"r1_{b}_{jq}_{hl}")
                    nc.sync.dma_start(out=r1, in_=rbf[r:r + 1, :])
                    ps_b = aux_pool.tile([HD, NB], F32, tag="aux",
                                         name=f"psb{b}_{jq}_{hl}")
                    nc.tensor.matmul(ps_b, ones_sb, r1, start=True, stop=True)
                    cnt = cn_pool.tile([HD, NB], BF16, tag="cn", name=f"cn{b}_{jq}_{hl}")
                    nc.vector.tensor_mul(cnt, cm[0:HD, :], ps_b)
                    for h in range(2):  # two dest cores per jq block
                        dcore = 2 * jq + h
                        nc.sync.dma_start(
                            out=bass.AP(a2a_in[b], dcore * GW * QH + hl * HD * QH,
                                        [[QH, HD], [1, QH]]),
                            in_=cnt[:, h * QH:(h + 1) * QH],
                        )

            def attention(b, fills, start_at=0):
                def pop_fills(n):
                    for _ in range(n):
                        if fills:
                            fills.popleft()()

                total_iters = 4 * KC
                it = 0
                for jq in range(4):
                    pc = {}
                    for hl in range(HPC):
                        pc[hl] = pc_pool.tile([HD + 1, NB], F32, tag="pc",
                                              name=f"pc{b}_{jq}_{hl}")
                    q0 = qT_sb[b][0:HD, ts(jq, NB)]
                    q1 = qT_sb[b][HD:2 * HD, ts(jq, NB)]
                    vs = v_sb[b]
                    for kc2 in range(KC // 2):
                        # two kc chunks share one exp activation (FD=2048)
                        et = et_pool.tile([128, 2, 2, NB], BF16, tag="et")
                        sT = sq_pool.tile([128, 2, 2, NB], BF16, tag="sq")
                        for ki in range(2):
                            kc = 2 * kc2 + ki
                            ft = ft_pool.tile([128, NB], BF16, tag="ft",
                                              name=f"ft{b}_{jq}_{kc}")
                            nc.sync.dma_start(
                                out=ft,
                                in_=bass.AP(fT_h, b * T * T + kc * 128 * T + jq * NB,
                                            [[T, 128], [1, NB]]),
                            )
                            ps_s = ps2_pool.tile([128, 2, NB], F32, tag="ps2")
                            nc.tensor.matmul(ps_s[:, 0, :], kT_sb[b][0:HD, ts(kc, 128)],
                                             q0, start=True, stop=True)
                            nc.tensor.matmul(ps_s[:, 1, :],
                                             kT_sb[b][HD:2 * HD, ts(kc, 128)],
                                             q1, start=True, stop=True)
                            ft_bc = bass.AP(ft.tensor, ft.offset,
                                            [ft.ap[0], [0, 2], [1, NB]])
                            nc.vector.tensor_mul(sT[:, ki, :, :], ps_s, ft_bc)
                        nc.scalar.activation(et, sT, AF.Exp)
                        for ki in range(2):
                            kc = 2 * kc2 + ki
                            for hl in range(HPC):
                                c0 = hl * (HD + 1)
                                nc.tensor.matmul(
                                    pc[hl], vs[:, kc, c0:c0 + HD + 1],
                                    et[:, ki, hl, :],
                                    start=(kc == 0), stop=(kc == KC - 1),
                                )
                            it += 1
                            # pace the fill queue across remaining iterations
                            if it >= start_at:
                                rem = total_iters - it
                                n = len(fills) if rem == 0 else -(-len(fills) // rem)
                                pop_fills(min(n, 2))
                    for hl in range(HPC):
                        cm = cm_pool.tile([HD + 1, NB], F32, tag="cm",
                                          name=f"cm{b}_{jq}_{hl}")
                        nc.scalar.activation(cm, pc[hl], AF.Copy)
                        ctx_cm[(jq, hl)] = cm
                    if jq in (1, 3):
                        emit_norm_pair(b, jq // 2)

            # ---------------- tail emitters (per phase) ----------------
            wo_sb = bo_sb = gamma_bc = beta_bc = ident = None

            def emit_tail_consts():
                nonlocal wo_sb, bo_sb, gamma_bc, beta_bc, ident
                wo_sb = consts.tile([128, DC, D], BF16, tag="wo")
                nc.sync.dma_start(out=wo_sb, in_=bass.AP(
                    wo_h, 0, [[D, 128], [128 * D, DC], [1, D]]))
                bo_sb = consts.tile([128, DC], F32, tag="bo")
                nc.sync.dma_start(out=bo_sb, in_=bo_h[:, :])
                gamma_bc = consts.tile([128, D], F32, tag="gamma")
                nc.sync.dma_start(out=gamma_bc, in_=bass.AP(gamma_h, 0, [[0, 128], [1, D]]))
                beta_bc = consts.tile([128, D], F32, tag="beta")
                nc.sync.dma_start(out=beta_bc, in_=bass.AP(beta_h, 0, [[0, 128], [1, D]]))
                ident = consts.tile([128, 128], BF16, tag="ident")
                make_identity(nc, ident)

            def emit_phase_closures(b):
                """Out-proj + residual + LN for phase b (q rows c*QH..): closures."""
                closures = []
                qres_t, ctxT, outT = [], [], []

                def start():
                    for qc in range(NQC):
                        qt = qres_pool.tile([128, D], F32, tag="qres",
                                            name=f"qres{b}_{qc}")
                        nc.sync.dma_start(out=qt, in_=qres_h[b][qc * 128:(qc + 1) * 128, :])
                        qres_t.append(qt)
                    for r in range(N_CORES):
                        ct = ctxt_pool.tile([GW, QH], BF16, tag="ctxT",
                                            name=f"ctxT{b}_{r}")
                        nc.sync.dma_start(
                            out=ct, in_=bass.AP(a2a_out[b], r * GW * QH, [[QH, GW], [1, QH]]))
                        ctxT.append(ct)

                closures.append(start)
                for dm in range(DC):
                    ps_ref = []

                    def mk_mm(kc, dm=dm, ps_ref=ps_ref):
                        def f():
                            if kc == 0:
                                ps_ref.append(aux_pool.tile(
                                    [128, NB], F32, tag="aux", name=f"pso{b}_{dm}"))
                            nc.tensor.matmul(
                                ps_ref[0][:, 0:QH], wo_sb[:, kc, ts(dm, 128)],
                                ctxT[kc], start=(kc == 0), stop=(kc == DC - 1))
                        return f

                    for kc in range(DC):
                        closures.append(mk_mm(kc))

                    def mk_bias(dm=dm, ps_ref=ps_ref):
                        def f():
                            ot = outt_pool.tile([128, QH], BF16, tag="outT",
                                                name=f"outT{b}_{dm}")
                            nc.scalar.activation(ot, ps_ref[0][:, 0:QH], AF.Identity,
                                                 bias=bo_sb[:, dm:dm + 1])
                            outT.append(ot)
                        return f

                    closures.append(mk_bias())

                for qc in range(NQC):
                    st_ref = []

                    def mk_tr(di, qc=qc, st_ref=st_ref):
                        def f():
                            if di == 0:
                                st_ref.append(aux_pool.tile(
                                    [128, D], BF16, tag="aux", name=f"pst{b}_{qc}"))
                            nc.tensor.transpose(
                                st_ref[0][:, di * 128:(di + 1) * 128],
                                outT[di][:, qc * 128:(qc + 1) * 128], ident)
                        return f

                    for di in range(DC):
                        closures.append(mk_tr(di))

                    def mk_ln(qc=qc, st_ref=st_ref):
                        def f():
                            resid = tail_pool.tile([128, D], F32, tag="resid")
                            stats = tail_pool.tile([128, 2, 6], F32, tag="stats")
                            mv = tail_pool.tile([128, 2], F32, tag="mv")
                            rstd = tail_pool.tile([128, 1], F32, tag="rstd")
                            nc.vector.tensor_add(resid, st_ref[0], qres_t[qc])
                            for half in range(2):
                                nc.vector.bn_stats(
                                    stats[:, half, :], resid[:, half * 512:(half + 1) * 512])
                            nc.vector.bn_aggr(mv, stats)
                            nc.scalar.activation(rstd, mv[:, 1:2], AF.Sqrt,
                                                 bias=eps_sb[:, :])
                            nc.vector.reciprocal(rstd, rstd)
                            outn = tail_pool.tile([128, D], F32, tag="outn")
                            nc.vector.tensor_scalar(outn, resid, mv[:, 0:1], rstd,
                                                    op0=ALU.subtract, op1=ALU.mult)
                            if qc % 2 == 0:
                                nc.vector.tensor_mul(outn, outn, gamma_bc)
                                nc.gpsimd.tensor_add(outn, outn, beta_bc)
                            else:
                                nc.gpsimd.tensor_mul(outn, outn, gamma_bc)
                                nc.vector.tensor_add(outn, outn, beta_bc)
                            nc.sync.dma_start(
                                out=out_h[b][qc * 128:(qc + 1) * 128, :], in_=outn)
                        return f

                    closures.append(mk_ln())
                return closures

            # ---------------- main schedule ----------------
            # batch 0 projections, emitted dense (PE warms up here)
            dma0, cl0 = emit_proj_closures(0)
            dma0()
            for f in cl0:
                f()
            # batch 1 projections interleaved into batch-0 attention
            dma1, cl1 = emit_proj_closures(1)
            dma1()
            fills = deque(cl1)
            attention(0, fills)
            while fills:
                fills.popleft()()
            nc.gpsimd.collective_compute(
                "AllToAll", ALU.bypass,
                ins=[a2a_in[0][:, :, :].opt()],
                outs=[a2a_out[0][:, :, :].opt()],
                replica_groups=[list(range(N_CORES))],
            )
            emit_tail_consts()
            attention(1, deque())
            # phase-0 tail emitted here: its ctxT DMAs wait on the (long done)
            # a2a#0 without blocking any attention DMA, and its compute
            # overlaps the a2a#1 collective flight below.
            for f in emit_phase_closures(0):
                f()
            nc.gpsimd.collective_compute(
                "AllToAll", ALU.bypass,
                ins=[a2a_in[1][:, :, :].opt()],
                outs=[a2a_out[1][:, :, :].opt()],
                replica_groups=[list(range(N_CORES))],
            )
            for f in emit_phase_closures(1):
                f()

    nc.compile()
    return nc


# ---------------- host side ----------------

def _prep_inputs(query, key_in, mask, Wq, bq, Wk, bk, Wv, bv, Wo, bo, gamma, beta):
    bf = ml_dtypes.bfloat16
    Bv, Tv, Dv = query.shape
    qT = np.ascontiguousarray(np.transpose(query.astype(np.float32), (0, 2, 1))).astype(bf)
    kT = np.ascontiguousarray(np.transpose(key_in.astype(np.float32), (0, 2, 1))).astype(bf)
    m = mask.reshape(Bv, Tv, Tv).astype(np.float32)
    fT = np.ascontiguousarray(np.transpose(0.0625 * m + 0.0625, (0, 2, 1))).astype(bf)
    # fold v-bias into out-proj bias: softmax rows sum to 1
    bo_f = (bo.astype(np.float64) + bv.astype(np.float64) @ Wo.astype(np.float64)
            ).astype(np.float32)
    in_maps = []
    for c in range(N_CORES):
        h0 = HPC * c
        cols = slice(h0 * HD, (h0 + HPC) * HD)
        im = {
            "qT": qT,
            "kT": kT,
            "fT": fT,
            "wq": np.ascontiguousarray(Wq[:, cols]).astype(bf),
            "wk": np.ascontiguousarray(Wk[:, cols]).astype(bf),
            "wv": np.ascontiguousarray(Wv[:, cols]).astype(bf),
            "wo": Wo.astype(bf),
            "bq": np.ascontiguousarray(bq[cols]).reshape(GW, 1).astype(np.float32),
            "bk": np.ascontiguousarray(bk[cols]).reshape(GW, 1).astype(np.float32),
            "bo": np.ascontiguousarray(bo_f.reshape(DC, 128).T).astype(np.float32),
            "gamma": gamma.astype(np.float32),
            "beta": beta.astype(np.float32),
        }
        for b in range(B):
            im[f"qres{b}"] = np.ascontiguousarray(
                query[b, c * QH:(c + 1) * QH, :]).astype(np.float32)
        in_maps.append(im)
    return in_maps


def _run(inputs, trace=False):
    key = "nc"
    if key not in _cached:
        _cached[key] = build_kernel()
    nc = _cached[key]
    in_maps = _prep_inputs(**inputs)
    res = run_bass_kernel_spmd(nc, in_maps, core_ids=list(range(N_CORES)), trace=trace)
    out = np.zeros((B, T, D), np.float32)
    for c in range(N_CORES):
        for b in range(B):
            out[b, c * QH:(c + 1) * QH, :] = res.results[c][f"out{b}"]
    return out, res


def _norm_inputs(inputs):
    np_inputs = {k: np.asarray(v) for k, v in inputs.items()}
    if "key" in np_inputs and "key_in" not in np_inputs:
        np_inputs["key_in"] = np_inputs.pop("key")
    return np_inputs


def kernel(**inputs):
    out, _ = _run(_norm_inputs(inputs), trace=False)
    return out


def kernel_traced(**inputs):
    return _run(_norm_inputs(inputs), trace=True)


# revision 23
# speedup vs baseline: 1.0141x; 1.0141x over previous
"""Bass/Trainium2 kernel for nn_CrossAttentionBlock (B=2, T=2048, D=1024, H=16).

Sharding: 8 cores; core c owns heads {2c, 2c+1} for BOTH batches (tensor
parallel over heads).  Output rows: core c owns q-rows [c*256,(c+1)*256) of
BOTH batches; one AllToAll per batch redistributes per-head context, the
first one fully overlapped with batch-1 attention.

Math notes:
  - masked softmax logits: s_eff = s*(1+m)/16 with m in {0,1}
    => unnormalized weights et = select(m, E^2, E) with E = exp(s/16).
    E computed by ScalarE directly from PSUM (scale=1/16 free); E^2 on
    DVE/GPSIMD (bf16 SBUF 2x); select via DVE copy_predicated(mask=m).
  - clamp(+-50) is a provable no-op for these inputs (|s_eff| < ~7).
  - softmax denominator accumulated via a ones-column appended to v.
  - v-bias bv folded into the out-proj bias on host:
    bo' = bo + Wo^T bv  (softmax rows sum to 1).
  - reciprocal via reciprocal_approx_fast; broadcast via DMA (stride-0).
"""

import sys

sys.path.insert(0, "/opt/trn_rl_repo")

from collections import deque
from contextlib import ExitStack

import numpy as np
import ml_dtypes

import concourse.bass as bass
import concourse.mybir as mybir
import concourse.tile as tile
from concourse import bacc
from concourse import tile_utils
from concourse.bass_utils import run_bass_kernel_spmd
from concourse.masks import make_identity

tile_utils.max_sbuf_usage = 204 * 1024

BF16 = mybir.dt.bfloat16
F32 = mybir.dt.float32
AF = mybir.ActivationFunctionType
ALU = mybir.AluOpType
ts = bass.ts

N_CORES = 8
B, D, H = 2, 1024, 16
T = 2048
HD = D // H              # 64 head dim
HPC = H // N_CORES       # 2 heads per core
GW = HPC * HD            # 128 head-group width per core
DC = D // 128            # 8 d chunks
KC = T // 128            # 16 k chunks
NB = 512                 # proj moving N
QH = T // N_CORES        # 256 output q rows per core per batch
NQC = QH // 128          # 2

USE_SELECT = True        # select-path masking (vs baseline fp32 mul path)

_cached = {}


def build_kernel():
    nc = bacc.Bacc(None, num_devices=N_CORES)

    qT_h = nc.dram_tensor("qT", [B, D, T], BF16, kind="ExternalInput")
    kT_h = nc.dram_tensor("kT", [B, D, T], BF16, kind="ExternalInput")
    fT_h = nc.dram_tensor("fT", [B, T, T], BF16, kind="ExternalInput")
    wq_h = nc.dram_tensor("wq", [D, GW], BF16, kind="ExternalInput")
    wk_h = nc.dram_tensor("wk", [D, GW], BF16, kind="ExternalInput")
    wv_h = nc.dram_tensor("wv", [D, GW], BF16, kind="ExternalInput")
    wo_h = nc.dram_tensor("wo", [D, D], BF16, kind="ExternalInput")
    bq_h = nc.dram_tensor("bq", [GW, 1], F32, kind="ExternalInput")
    bk_h = nc.dram_tensor("bk", [GW, 1], F32, kind="ExternalInput")
    bo_h = nc.dram_tensor("bo", [128, DC], F32, kind="ExternalInput")
    gamma_h = nc.dram_tensor("gamma", [D], F32, kind="ExternalInput")
    beta_h = nc.dram_tensor("beta", [D], F32, kind="ExternalInput")
    qres_h = {}
    out_h = {}
    for b in range(B):
        qres_h[b] = nc.dram_tensor(f"qres{b}", [QH, D], F32, kind="ExternalInput")
        out_h[b] = nc.dram_tensor(f"out{b}", [QH, D], F32, kind="ExternalOutput")

    a2a_in = [nc.dram_tensor(f"a2a_in{b}", [N_CORES, GW, QH], BF16) for b in range(B)]
    a2a_out = [nc.dram_tensor(f"a2a_out{b}", [N_CORES, GW, QH], BF16) for b in range(B)]

    with tile.TileContext(nc) as tc:
        with (
            tc.tile_pool(name="consts", bufs=1) as consts,
            tc.tile_pool(name="ps2", bufs=2, space="PSUM") as ps2_pool,
            tc.tile_pool(name="pspc", bufs=2, space="PSUM") as pc_pool,
            tc.tile_pool(name="psaux", bufs=2, space="PSUM") as aux_pool,
            tc.tile_pool(name="xt", bufs=4) as xt_pool,
            tc.tile_pool(name="qk", bufs=2) as qk_pool,
            tc.tile_pool(name="vpool", bufs=2) as v_pool,
            tc.tile_pool(name="ft", bufs=22) as ft_pool,
            tc.tile_pool(name="et", bufs=4) as et_pool,
            tc.tile_pool(name="sq", bufs=3) as sq_pool,
            tc.tile_pool(name="cm", bufs=4) as cm_pool,
            tc.tile_pool(name="cn", bufs=4) as cn_pool,
            tc.tile_pool(name="sums", bufs=2) as sums_pool,
            tc.tile_pool(name="rb", bufs=4) as rb_pool,
            tc.tile_pool(name="qres", bufs=2) as qres_pool,
            tc.tile_pool(name="ctxt", bufs=8) as ctxt_pool,
            tc.tile_pool(name="outt", bufs=8) as outt_pool,
            tc.tile_pool(name="tail", bufs=2) as tail_pool,
        ):
            # ---------------- constants (startup set) ----------------
            wq_sb = consts.tile([128, DC, GW], BF16, tag="wq")
            nc.sync.dma_start(out=wq_sb, in_=bass.AP(wq_h, 0, [[GW, 128], [128 * GW, DC], [1, GW]]))
            wk_sb = consts.tile([128, DC, GW], BF16, tag="wk")
            nc.sync.dma_start(out=wk_sb, in_=bass.AP(wk_h, 0, [[GW, 128], [128 * GW, DC], [1, GW]]))
            wv_sb = consts.tile([128, DC, GW], BF16, tag="wv")
            nc.sync.dma_start(out=wv_sb, in_=bass.AP(wv_h, 0, [[GW, 128], [128 * GW, DC], [1, GW]]))
            bq_sb = consts.tile([GW, 1], F32, tag="bq")
            nc.sync.dma_start(out=bq_sb, in_=bq_h[:, :])
            bk_sb = consts.tile([GW, 1], F32, tag="bk")
            nc.sync.dma_start(out=bk_sb, in_=bk_h[:, :])
            eps_sb = consts.tile([128, 1], F32, tag="eps")
            nc.vector.memset(eps_sb, 1e-5)
            ones_sb = consts.tile([1, HD], BF16, tag="ones")
            nc.vector.memset(ones_sb, 1.0)

            qT_sb, kT_sb, v_sb = {}, {}, {}

            # ---------------- projection emitters ----------------
            def emit_proj_closures(b):
                """Returns (dma_fn, list-of-closures) for batch b's q/k/v proj."""
                qT_sb[b] = qk_pool.tile([GW, T], BF16, tag="qT", name=f"qT{b}")
                kT_sb[b] = qk_pool.tile([GW, T], BF16, tag="kT", name=f"kT{b}")
                vt = v_pool.tile([128, KC, HPC * (HD + 1)], BF16, tag="v", name=f"v{b}")
                v_sb[b] = vt
                xtq, xtk = [], []

                def dmas():
                    for half in range(4):
                        xh = xt_pool.tile([128, 2, T], BF16, tag="xt", name=f"xtq{b}_{half}")
                        nc.sync.dma_start(
                            out=xh,
                            in_=bass.AP(qT_h, b * D * T + half * 2 * 128 * T,
                                        [[T, 128], [128 * T, 2], [1, T]]),
                        )
                        xtq.append(xh)
                    for half in range(4):
                        xh = xt_pool.tile([128, 2, T], BF16, tag="xt", name=f"xtk{b}_{half}")
                        nc.sync.dma_start(
                            out=xh,
                            in_=bass.AP(kT_h, b * D * T + half * 2 * 128 * T,
                                        [[T, 128], [128 * T, 2], [1, T]]),
                        )
                        xtk.append(xh)

                closures = []
                # q and k projections: per nb, accumulate 8 kc into one aux bank
                for which, xts, w_sb, b_sb, dst in (
                    ("q", xtq, wq_sb, bq_sb, "qT"),
                    ("k", xtk, wk_sb, bk_sb, "kT"),
                ):
                    for nb in range(T // NB):
                        ps_ref = []

                        def mk_mm(kc, nb=nb, xts=xts, w_sb=w_sb, ps_ref=ps_ref,
                                  which=which):
                            def f():
                                if kc == 0:
                                    ps_ref.append(aux_pool.tile(
                                        [128, NB], F32, tag="aux",
                                        name=f"pj{which}{b}_{nb}"))
                                nc.tensor.matmul(
                                    ps_ref[0],
                                    w_sb[:, kc, :],
                                    xts[kc // 2][:, kc % 2, ts(nb, NB)],
                                    start=(kc == 0), stop=(kc == DC - 1),
                                )
                            return f

                        for kc in range(DC):
                            closures.append(mk_mm(kc))

                        def mk_bias(nb=nb, ps_ref=ps_ref, b_sb=b_sb, dst=dst):
                            def f():
                                tgt = qT_sb[b] if dst == "qT" else kT_sb[b]
                                nc.scalar.activation(
                                    tgt[:, ts(nb, NB)], ps_ref[0], AF.Identity,
                                    bias=b_sb[:, :])
                            return f

                        closures.append(mk_bias())

                # v projection: per mc, accumulate 8 kc, then DVE copy (interleaved)
                for mc in range(KC):
                    psv_ref = []

                    def mk_vmm(kc, mc=mc, psv_ref=psv_ref):
                        def f():
                            if kc == 0:
                                psv_ref.append(aux_pool.tile(
                                    [128, NB], F32, tag="aux", name=f"pjv{b}_{mc}"))
                            nc.tensor.matmul(
                                psv_ref[0][:, 0:GW],
                                xtk[kc // 2][:, kc % 2, ts(mc, 128)],
                                wv_sb[:, kc, :],
                                start=(kc == 0), stop=(kc == DC - 1),
                            )
                        return f

                    for kc in range(DC):
                        closures.append(mk_vmm(kc))

                    def mk_vcopy(mc=mc, psv_ref=psv_ref, vt=vt):
                        def f():
                            vslot = vt[:, mc, :]
                            dst = bass.AP(vslot.tensor, vslot.offset,
                                          [vslot.ap[0], [HD + 1, HPC], [1, HD]])
                            psl = psv_ref[0][:, 0:GW]
                            src = bass.AP(psl.tensor, psl.offset,
                                          [psl.ap[0], [HD, HPC], [1, HD]])
                            nc.vector.tensor_copy(dst, src)
                        return f

                    closures.append(mk_vcopy())

                def mk_ones(vt=vt):
                    def f():
                        nc.vector.memset(vt[:, :, HD:HD + 1], 1.0)
                        nc.vector.memset(vt[:, :, 2 * HD + 1:2 * HD + 2], 1.0)
                    return f

                closures.append(mk_ones())
                return dmas, closures

            # ---------------- attention loop ----------------
            ctx_cm = {}

            def emit_norm_pair(b, jqp):
                """Normalize context of jq pair (2*jqp, 2*jqp+1) and DMA to a2a_in."""
                su = sums_pool.tile([4, NB], F32, tag="sums", name=f"su{b}_{jqp}")
                for r, (jq, hl) in enumerate(
                        (jq, hl) for jq in (2 * jqp, 2 * jqp + 1) for hl in range(HPC)):
                    cm = ctx_cm[(jq, hl)]
                    nc.sync.dma_start(out=su[r:r + 1, :], in_=cm[HD:HD + 1, :])
                rc = sums_pool.tile([4, NB], F32, tag="rc", name=f"rc{b}_{jqp}")
                nc.vector.reciprocal_approx_fast(out=rc, in_=su)
                rbf = sums_pool.tile([4, NB], BF16, tag="rbf", name=f"rbf{b}_{jqp}")
                nc.vector.tensor_copy(rbf, rc)
                for r, (jq, hl) in enumerate(
                        (jq, hl) for jq in (2 * jqp, 2 * jqp + 1) for hl in range(HPC)):
                    cm = ctx_cm[(jq, hl)]
                    r1 = rb_pool.tile([1, NB], BF16, tag="r1", name=f"r1_{b}_{jq}_{hl}")
                    nc.sync.dma_start(out=r1, in_=rbf[r:r + 1, :])
                    ps_b = aux_pool.tile([HD, NB], F32, tag="aux",
                                         name=f"psb{b}_{jq}_{hl}")
                    nc.tensor.matmul(ps_b, ones_sb, r1, start=True, stop=True)
                    cnt = cn_pool.tile([HD, NB], BF16, tag="cn", name=f"cn{b}_{jq}_{hl}")
                    nc.vector.tensor_mul(cnt, cm[0:HD, :], ps_b)
                    for h in range(2):  # two dest cores per jq block
                        dcore = 2 * jq + h
                        nc.sync.dma_start(
                            out=bass.AP(a2a_in[b], dcore * GW * QH + hl * HD * QH,
                                        [[QH, HD], [1, QH]]),
                            in_=cnt[:, h * QH:(h + 1) * QH],
                        )

            def attention(b, fills, start_at=0):
                def pop_fills(n):
                    for _ in range(n):
                        if fills:
                            fills.popleft()()

                total_iters = 4 * KC
                it = 0
                for jq in range(4):
                    pc = {}
                    for hl in range(HPC):
                        pc[hl] = pc_pool.tile([HD + 1, NB], F32, tag="pc",
                                              name=f"pc{b}_{jq}_{hl}")
                    q0 = qT_sb[b][0:HD, ts(jq, NB)]
                    q1 = qT_sb[b][HD:2 * HD, ts(jq, NB)]
                    vs = v_sb[b]
                    for kc2 in range(KC // 2):
                        # two kc chunks share one exp activation (FD=2048)
                        et = et_pool.tile([128, 2, 2, NB], BF16, tag="et")
                        sT = sq_pool.tile([128, 2, 2, NB], BF16, tag="sq")
                        for ki in range(2):
                            kc = 2 * kc2 + ki
                            ft = ft_pool.tile([128, NB], BF16, tag="ft",
                                              name=f"ft{b}_{jq}_{kc}")
                            nc.sync.dma_start(
                                out=ft,
                                in_=bass.AP(fT_h, b * T * T + kc * 128 * T + jq * NB,
                                            [[T, 128], [1, NB]]),
                            )
                            ps_s = ps2_pool.tile([128, 2, NB], F32, tag="ps2")
                            nc.tensor.matmul(ps_s[:, 0, :], kT_sb[b][0:HD, ts(kc, 128)],
                                             q0, start=True, stop=True)
                            nc.tensor.matmul(ps_s[:, 1, :],
                                             kT_sb[b][HD:2 * HD, ts(kc, 128)],
                                             q1, start=True, stop=True)
                            ft_bc = bass.AP(ft.tensor, ft.offset,
                                            [ft.ap[0], [0, 2], [1, NB]])
                            nc.vector.tensor_mul(sT[:, ki, :, :], ps_s, ft_bc)
                        nc.scalar.activation(et, sT, AF.Exp)
                        for ki in range(2):
                            kc = 2 * kc2 + ki
                            for hl in range(HPC):
                                c0 = hl * (HD + 1)
                                nc.tensor.matmul(
                                    pc[hl], vs[:, kc, c0:c0 + HD + 1],
                                    et[:, ki, hl, :],
                                    start=(kc == 0), stop=(kc == KC - 1),
                                )
                            it += 1
                            # pace the fill queue across remaining iterations
                            if it >= start_at:
                                rem = total_iters - it
                                n = len(fills) if rem == 0 else -(-len(fills) // rem)
                                pop_fills(min(n, 3))
                    for hl in range(HPC):
                        cm = cm_pool.tile([HD + 1, NB], F32, tag="cm",
                                          name=f"cm{b}_{jq}_{hl}")
                        nc.scalar.activation(cm, pc[hl], AF.Copy)
                        ctx_cm[(jq, hl)] = cm
                    if jq in (1, 3):
                        emit_norm_pair(b, jq // 2)

            # ---------------- tail emitters (per phase) ----------------
            wo_sb = bo_sb = gamma_bc = beta_bc = ident = None

            def emit_tail_consts():
                nonlocal wo_sb, bo_sb, gamma_bc, beta_bc, ident
                wo_sb = consts.tile([128, DC, D], BF16, tag="wo")
                nc.sync.dma_start(out=wo_sb, in_=bass.AP(
                    wo_h, 0, [[D, 128], [128 * D, DC], [1, D]]))
                bo_sb = consts.tile([128, DC], F32, tag="bo")
                nc.sync.dma_start(out=bo_sb, in_=bo_h[:, :])
                gamma_bc = consts.tile([128, D], F32, tag="gamma")
                nc.sync.dma_start(out=gamma_bc, in_=bass.AP(gamma_h, 0, [[0, 128], [1, D]]))
                beta_bc = consts.tile([128, D], F32, tag="beta")
                nc.sync.dma_start(out=beta_bc, in_=bass.AP(beta_h, 0, [[0, 128], [1, D]]))
                ident = consts.tile([128, 128], BF16, tag="ident")
                make_identity(nc, ident)

            def emit_phase_closures(b):
                """Out-proj + residual + LN for phase b (q rows c*QH..): closures."""
                closures = []
                qres_t, ctxT, outT = [], [], []

                def start():
                    for qc in range(NQC):
                        qt = qres_pool.tile([128, D], F32, tag="qres",
                                            name=f"qres{b}_{qc}")
                        nc.sync.dma_start(out=qt, in_=qres_h[b][qc * 128:(qc + 1) * 128, :])
                        qres_t.append(qt)
                    for r in range(N_CORES):
                        ct = ctxt_pool.tile([GW, QH], BF16, tag="ctxT",
                                            name=f"ctxT{b}_{r}")
                        nc.sync.dma_start(
                            out=ct, in_=bass.AP(a2a_out[b], r * GW * QH, [[QH, GW], [1, QH]]))
                        ctxT.append(ct)

                closures.append(start)
                for dm in range(DC):
                    ps_ref = []

                    def mk_mm(kc, dm=dm, ps_ref=ps_ref):
                        def f():
                            if kc == 0:
                                ps_ref.append(aux_pool.tile(
                                    [128, NB], F32, tag="aux", name=f"pso{b}_{dm}"))
                            nc.tensor.matmul(
                                ps_ref[0][:, 0:QH], wo_sb[:, kc, ts(dm, 128)],
                                ctxT[kc], start=(kc == 0), stop=(kc == DC - 1))
                        return f

                    for kc in range(DC):
                        closures.append(mk_mm(kc))

                    def mk_bias(dm=dm, ps_ref=ps_ref):
                        def f():
                            ot = outt_pool.tile([128, QH], BF16, tag="outT",
                                                name=f"outT{b}_{dm}")
                            nc.scalar.activation(ot, ps_ref[0][:, 0:QH], AF.Identity,
                                                 bias=bo_sb[:, dm:dm + 1])
                            outT.append(ot)
                        return f

                    closures.append(mk_bias())

                for qc in range(NQC):
                    st_ref = []

                    def mk_tr(di, qc=qc, st_ref=st_ref):
                        def f():
                            if di == 0:
                                st_ref.append(aux_pool.tile(
                                    [128, D], BF16, tag="aux", name=f"pst{b}_{qc}"))
                            nc.tensor.transpose(
                                st_ref[0][:, di * 128:(di + 1) * 128],
                                outT[di][:, qc * 128:(qc + 1) * 128], ident)
                        return f

                    for di in range(DC):
                        closures.append(mk_tr(di))

                    def mk_ln(qc=qc, st_ref=st_ref):
                        def f():
                            resid = tail_pool.tile([128, D], F32, tag="resid")
                            stats = tail_pool.tile([128, 2, 6], F32, tag="stats")
                            mv = tail_pool.tile([128, 2], F32, tag="mv")
                            rstd = tail_pool.tile([128, 1], F32, tag="rstd")
                            nc.vector.tensor_add(resid, st_ref[0], qres_t[qc])
                            for half in range(2):
                                nc.vector.bn_stats(
                                    stats[:, half, :], resid[:, half * 512:(half + 1) * 512])
                            nc.vector.bn_aggr(mv, stats)
                            nc.scalar.activation(rstd, mv[:, 1:2], AF.Sqrt,
                                                 bias=eps_sb[:, :])
                            nc.vector.reciprocal(rstd, rstd)
                            outn = tail_pool.tile([128, D], F32, tag="outn")
                            nc.vector.tensor_scalar(outn, resid, mv[:, 0:1], rstd,
                                                    op0=ALU.subtract, op1=ALU.mult)
                            if qc % 2 == 0:
                                nc.vector.tensor_mul(outn, outn, gamma_bc)
                                nc.gpsimd.tensor_add(outn, outn, beta_bc)
                            else:
                                nc.gpsimd.tensor_mul(outn, outn, gamma_bc)
                                nc.vector.tensor_add(outn, outn, beta_bc)
                            nc.sync.dma_start(
                                out=out_h[b][qc * 128:(qc + 1) * 128, :], in_=outn)
                        return f

                    closures.append(mk_ln())
                return closures

            # ---------------- main schedule ----------------
            # batch 0 projections, emitted dense (PE warms up here)
            dma0, cl0 = emit_proj_closures(0)
            dma0()
            for f in cl0:
                f()
            # batch 1 projections interleaved into batch-0 attention
            dma1, cl1 = emit_proj_closures(1)
            dma1()
            fills = deque(cl1)
            attention(0, fills)
            while fills:
                fills.popleft()()
            nc.gpsimd.collective_compute(
                "AllToAll", ALU.bypass,
                ins=[a2a_in[0][:, :, :].opt()],
                outs=[a2a_out[0][:, :, :].opt()],
                replica_groups=[list(range(N_CORES))],
            )
            emit_tail_consts()
            attention(1, deque())
            # phase-0 tail emitted here: its ctxT DMAs wait on the (long done)
            # a2a#0 without blocking any attention DMA, and its compute
            # overlaps the a2a#1 collective flight below.
            for f in emit_phase_closures(0):
                f()
            nc.gpsimd.collective_compute(
                "AllToAll", ALU.bypass,
                ins=[a2a_in[1][:, :, :].opt()],
                outs=[a2a_out[1][:, :, :].opt()],
                replica_groups=[list(range(N_CORES))],
            )
            for f in emit_phase_closures(1):
                f()

    nc.compile()
    return nc


# ---------------- host side ----------------

def _prep_inputs(query, key_in, mask, Wq, bq, Wk, bk, Wv, bv, Wo, bo, gamma, beta):
    bf = ml_dtypes.bfloat16
    Bv, Tv, Dv = query.shape
    qT = np.ascontiguousarray(np.transpose(query.astype(np.float32), (0, 2, 1))).astype(bf)
    kT = np.ascontiguousarray(np.transpose(key_in.astype(np.float32), (0, 2, 1))).astype(bf)
    m = mask.reshape(Bv, Tv, Tv).astype(np.float32)
    fT = np.ascontiguousarray(np.transpose(0.0625 * m + 0.0625, (0, 2, 1))).astype(bf)
    # fold v-bias into out-proj bias: softmax rows sum to 1
    bo_f = (bo.astype(np.float64) + bv.astype(np.float64) @ Wo.astype(np.float64)
            ).astype(np.float32)
    in_maps = []
    for c in range(N_CORES):
        h0 = HPC * c
        cols = slice(h0 * HD, (h0 + HPC) * HD)
        im = {
            "qT": qT,
            "kT": kT,
            "fT": fT,
            "wq": np.ascontiguousarray(Wq[:, cols]).astype(bf),
            "wk": np.ascontiguousarray(Wk[:, cols]).astype(bf),
            "wv": np.ascontiguousarray(Wv[:, cols]).astype(bf),
            "wo": Wo.astype(bf),
            "bq": np.ascontiguousarray(bq[cols]).reshape(GW, 1).astype(np.float32),
            "bk": np.ascontiguousarray(bk[cols]).reshape(GW, 1).astype(np.float32),
            "bo": np.ascontiguousarray(bo_f.reshape(DC, 128).T).astype(np.float32),
            "gamma": gamma.astype(np.float32),
            "beta": beta.astype(np.float32),
        }
        for b in range(B):
            im[f"qres{b}"] = np.ascontiguousarray(
                query[b, c * QH:(c + 1) * QH, :]).astype(np.float32)
        in_maps.append(im)
    return in_maps


def _run(inputs, trace=False):
    key = "nc"
    if key not in _cached:
        _cached[key] = build_kernel()
    nc = _cached[key]
    in_maps = _prep_inputs(**inputs)
    res = run_bass_kernel_spmd(nc, in_maps, core_ids=list(range(N_CORES)), trace=trace)
    out = np.zeros((B, T, D), np.float32)
    for c in range(N_CORES):
        for b in range(B):
            out[b, c * QH:(c + 1) * QH, :] = res.results[c][f"out{b}"]
    return out, res


def _norm_inputs(inputs):
    np_inputs = {k: np.asarray(v) for k, v in inputs.items()}
    if "key" in np_inputs and "key_in" not in np_inputs:
        np_inputs["key_in"] = np_inputs.pop("key")
    return np_inputs


def kernel(**inputs):
    out, _ = _run(_norm_inputs(inputs), trace=False)
    return out


def kernel_traced(**inputs):
    return _run(_norm_inputs(inputs), trace=True)
